# revision 3
# baseline (speedup 1.0000x reference)
"""Trainium2 Bass kernel for nn_DCTFeatureModel.

Math: the reference pipeline (3D DCT-II over [time-in-bin, H, W], mean over
DCT bins, full-receptive-field Conv3d, bias, LeakyReLU) is linear up to the
LeakyReLU, so everything folds into a single small matmul:

    feat[b,s,o] = LeakyReLU( sum_{c,t,i,j} x[b,s,c,t,i,j] * Weff[s,o,t,i,j]
                             + bias[s,o] )
    Weff[s,o,t,i,j] = (1/8) * sum_{f,p,q} Ct[f,t] Cs[p,i] Cs[q,j] W[s,o,f,p,q]

Weff is tiny (2*64*2048 floats) and computed on host. The device kernel is
memory-bound: stream x, reduce over the 8 DCT bins (c), then a
[128b x 2048k] @ [2048k x 64o] matmul per subwindow.

Device dataflow (per core, bf16): the host casts x to bf16 and lays each
core's shard out as 4 contiguous 2 MB blocks, one per (s, g) chunk-group:
block[kin=128, c*1024 + chin*128 + b]. Each block is ONE dma_start (2 MB at
~line rate, all on the sync HWDGE ring so blocks land strictly in order).
The c-reduction is a single strided DVE tensor_reduce per column-half
(innermost axis = c, stride 1024), writing bf16 z tiles whose 128-col
slices are directly the matmul lhsT chunks. PE does 32 accumulating bf16
matmuls + 2 rank-1 bias matmuls; LeakyReLU = max(v, 0.02v). Everything is
SBUF-resident (~77 KB/partition), so the DMA stream never stalls on
buffer recycling.

Sharding: pure data-parallel over batch, 1024/8 = 128 rows per core.
"""

from contextlib import ExitStack

import ml_dtypes
import numpy as np

import concourse.bacc as bacc
import concourse.tile as tile
from concourse import mybir
from concourse.bass_utils import run_bass_kernel_spmd

# Problem shapes (hardcoded per contract)
B = 1024
NCORES = 8
BS = B // NCORES          # 128 batch rows per core
NSW = 2                   # subwindows
NBINS = 8                 # DCT bins (mean-reduced)
NDCT = 32                 # time points per bin
HW = 8
NF = 64                   # conv output filters per subwindow
K = NDCT * HW * HW        # 2048 contraction elements per (s, c)
P = 128                   # partitions
NCHUNK = K // P           # 16 k-chunks of 128
NG = 2                    # chunk-groups per s
CPG = NCHUNK // NG        # 8 chunks per group
GW = CPG * P              # 1024 columns per group tile
NBLK = NSW * NG           # 4 (s, g) blocks
OUT_F = NSW * NF          # 128 output features
SLOPE = 0.02
HGW = GW // 2             # 512-column halves for reduce/matmul pipelining

F32 = mybir.dt.float32
BF16 = mybir.dt.bfloat16
NP_BF16 = ml_dtypes.bfloat16

_cached = None
last_results = None


def _dct2(N):
    n = np.arange(N, dtype=np.float64)
    k = np.arange(N, dtype=np.float64)
    return 2.0 * np.cos(np.pi * (2.0 * n[None, :] + 1.0) * k[:, None] / (2.0 * N))


def _kernel_body(tc, x, w, bias, out):
    """x: [NBLK, 128, NBINS*GW] bf16, block (s,g) laid [kin, (c, chin, b)]
    w: [P, NSW*NCHUNK*NF] bf16; bias: [1, OUT_F] bf16; out: [BS, OUT_F] f32"""
    nc = tc.nc
    with ExitStack() as ctx:
        const_pool = ctx.enter_context(tc.tile_pool(name="const", bufs=1))
        xpool = ctx.enter_context(tc.tile_pool(name="xp", bufs=NBLK))
        zpool = ctx.enter_context(tc.tile_pool(name="zp", bufs=NBLK))
        tpool = ctx.enter_context(tc.tile_pool(name="tp", bufs=2))
        opool = ctx.enter_context(tc.tile_pool(name="op", bufs=1))
        pft_pool = ctx.enter_context(tc.tile_pool(name="pft", bufs=1, space="PSUM"))

        # consts dispatched off the scalar engine so the x stream (sync ring)
        # starts at once and blocks land strictly in issue order
        w_sb = const_pool.tile([P, NSW * NCHUNK * NF], BF16)
        nc.scalar.dma_start(out=w_sb, in_=w)
        bias_sb = const_pool.tile([1, OUT_F], BF16)
        nc.scalar.dma_start(out=bias_sb, in_=bias)
        ones = const_pool.tile([1, P], BF16)
        nc.gpsimd.memset(ones, 1.0)

        out_sb = opool.tile([BS, OUT_F], F32)
        psum_feat = [
            pft_pool.tile([P, NF], F32, tag=f"feat{s}", name=f"psum_feat{s}")
            for s in range(NSW)
        ]

        xtiles = []
        for blk in range(NBLK):
            t = xpool.tile([P, NBINS * GW], BF16, tag="x", name=f"x{blk}")
            nc.sync.dma_start(out=t, in_=x[blk])
            xtiles.append(t)

        for blk in range(NBLK):
            s, g = divmod(blk, NG)
            # strided view [kin, m=(chin,b), c]: innermost axis walks the 8
            # c-slices (stride GW) so one DVE reduce per half sums the bins
            xv = xtiles[blk].rearrange("p (c m) -> p m c", c=NBINS)
            z = zpool.tile([P, GW], BF16, tag="z", name=f"z{blk}")
            for h in range(2):
                with nc.allow_low_precision(reason="bf16 bin-sum, tol 2e-2"):
                    nc.vector.tensor_reduce(
                        out=z[:, h * HGW:(h + 1) * HGW],
                        in_=xv[:, h * HGW:(h + 1) * HGW, :],
                        axis=mybir.AxisListType.X,
                        op=mybir.AluOpType.add,
                    )
                for j in range(CPG // 2):
                    chin = h * (CPG // 2) + j
                    ch = g * CPG + chin
                    nc.tensor.matmul(
                        psum_feat[s],
                        lhsT=z[:, chin * P:(chin + 1) * P],
                        rhs=w_sb[:, (s * NCHUNK + ch) * NF:(s * NCHUNK + ch + 1) * NF],
                        start=(ch == 0),
                        stop=False,
                    )
            if g == NG - 1:
                # bias via rank-1 matmul: ones[1, b].T @ bias[1, o]
                nc.tensor.matmul(
                    psum_feat[s],
                    lhsT=ones,
                    rhs=bias_sb[:, s * NF:(s + 1) * NF],
                    start=False,
                    stop=True,
                )
                # LeakyReLU(v) = max(v, slope*v)  (DVE: gpsimd can't read PSUM)
                eng = nc.vector
                tmp = tpool.tile([P, NF], F32, tag="lr", name=f"lr{s}")
                eng.tensor_scalar_mul(tmp, psum_feat[s], SLOPE)
                eng.tensor_max(
                    out=out_sb[:, s * NF:(s + 1) * NF], in0=psum_feat[s], in1=tmp
                )
                nc.scalar.dma_start(
                    out=out[:, s * NF:(s + 1) * NF],
                    in_=out_sb[:, s * NF:(s + 1) * NF],
                )


def _build():
    global _cached
    if _cached is not None:
        return _cached
    nc = bacc.Bacc(
        "TRN2",
        target_bir_lowering=False,
        debug=False,
        enable_asserts=False,
        num_devices=NCORES,
    )
    x_ap = nc.dram_tensor(
        "x", [NBLK, P, NBINS * GW], BF16, kind="ExternalInput"
    ).ap()
    w_ap = nc.dram_tensor("w", [P, NSW * NCHUNK * NF], BF16, kind="ExternalInput").ap()
    b_ap = nc.dram_tensor("bias", [1, OUT_F], BF16, kind="ExternalInput").ap()
    out_ap = nc.dram_tensor("out", [BS, OUT_F], F32, kind="ExternalOutput").ap()
    with tile.TileContext(nc, trace_sim=False) as tc:
        _kernel_body(tc, x_ap, w_ap, b_ap, out_ap)
    nc.compile()
    _cached = nc
    return nc


def kernel(x, W, b):
    global last_results
    assert x.shape == (B, 1, NSW * NBINS * NDCT, HW, HW), x.shape
    nc = _build()

    # Host-side folding of the DCT matrices into the conv weights (tiny).
    Ct = _dct2(NDCT)                       # [f, t]
    Cs = _dct2(HW)                         # [p, i]
    Weff = np.einsum(
        "ft,pi,qj,sofpq->sotij", Ct, Cs, Cs, W.astype(np.float64), optimize=True
    ) / float(NBINS)
    Weff_k = Weff.reshape(NSW, NF, K)      # [s, o, k]
    # device layout: w[p, s*NCHUNK*NF + ch*NF + o] = Weff_k[s, o, ch*128 + p]
    w_dev = np.ascontiguousarray(
        Weff_k.reshape(NSW, NF, NCHUNK, P).transpose(3, 0, 2, 1).reshape(P, NSW * NCHUNK * NF)
    ).astype(NP_BF16)
    bias_dev = np.ascontiguousarray(b.reshape(1, OUT_F)).astype(NP_BF16)

    x2 = x.reshape(B, NSW, NBINS, NG, CPG, P)  # (b, s, c, g, chin, kin)
    in_maps = []
    for i in range(NCORES):
        xs = x2[i * BS:(i + 1) * BS]
        # -> [s, g, kin, c, chin, b]: one contiguous 2 MB bf16 block per (s, g)
        xt = np.ascontiguousarray(xs.transpose(1, 3, 5, 2, 4, 0)).astype(NP_BF16)
        in_maps.append({
            "x": xt.reshape(NBLK, P, NBINS * GW),
            "w": w_dev,
            "bias": bias_dev,
        })
    res = run_bass_kernel_spmd(nc, in_maps, core_ids=list(range(NCORES)))
    last_results = res
    return np.concatenate([r["out"] for r in res.results], axis=0)


# revision 4
# speedup vs baseline: 1.7715x; 1.7715x over previous
"""Trainium2 Bass kernel for nn_DCTFeatureModel.

Math: the reference pipeline (3D DCT-II over [time-in-bin, H, W], mean over
DCT bins, full-receptive-field Conv3d, bias, LeakyReLU) is linear up to the
LeakyReLU, so everything folds into a single small matmul:

    feat[b,s,o] = LeakyReLU( sum_{c,t,i,j} x[b,s,c,t,i,j] * Weff[s,o,t,i,j]
                             + bias[s,o] )
    Weff[s,o,t,i,j] = (1/8) * sum_{f,p,q} Ct[f,t] Cs[p,i] Cs[q,j] W[s,o,f,p,q]

Weff is tiny (2*64*2048 floats) and computed on host. The device kernel is
memory-bound: stream x (bf16, 8.4 MB/core), reduce over the 8 DCT bins (c),
then a [128b x 2048k] @ [2048k x 64o] matmul per subwindow.

Device dataflow (per core): host casts x to bf16 in blocks
[kin=128, (c, chin, b)] per (s, g) chunk-group; each block streams as two
1 MB half-DMAs (c0-3 / c4-7) on the sync HWDGE ring so halves land in
strict order. The c-reduction is a pairwise binary tree of unit-stride
bf16 tensor_adds on DVE (2x packed mode, ~0.6 us per [128,1024] add),
pipelined against the DMA stream; the half-split lets 3 of the 7 adds per
block run while the second half is still in flight. Matmuls are
w-stationary: lhsT = Weff chunk [128k, 64o], rhs = z chunk [128k, 128b],
PSUM [64o, 128b] per subwindow. Bias + LeakyReLU fuse into ONE scalar
engine activation (Lrelu, per-partition bias) straight out of PSUM, and
the output lands as [s*64+o, b] which the host transposes for free.

Sharding: pure data-parallel over batch, 1024/8 = 128 rows per core.
"""

from contextlib import ExitStack

import ml_dtypes
import numpy as np

import concourse.bacc as bacc
import concourse.tile as tile
from concourse import mybir
from concourse.bass_utils import run_bass_kernel_spmd

# Problem shapes (hardcoded per contract)
B = 1024
NCORES = 8
BS = B // NCORES          # 128 batch rows per core
NSW = 2                   # subwindows
NBINS = 8                 # DCT bins (mean-reduced)
NDCT = 32                 # time points per bin
HW = 8
NF = 64                   # conv output filters per subwindow
K = NDCT * HW * HW        # 2048 contraction elements per (s, c)
P = 128                   # partitions
NCHUNK = K // P           # 16 k-chunks of 128
NG = 2                    # chunk-groups per s
CPG = NCHUNK // NG        # 8 chunks per group
GW = CPG * P              # 1024 columns per (c) slice of a group
NBLK = NSW * NG           # 4 (s, g) blocks
NHALF = 2 * NBLK          # 8 half-block DMAs
HCOLS = (NBINS // 2) * GW  # 4096 columns per half (4 c-slices)
OUT_F = NSW * NF          # 128 output features
SLOPE = 0.02

F32 = mybir.dt.float32
BF16 = mybir.dt.bfloat16
NP_BF16 = ml_dtypes.bfloat16

_cached = None
last_results = None


def _dct2(N):
    n = np.arange(N, dtype=np.float64)
    k = np.arange(N, dtype=np.float64)
    return 2.0 * np.cos(np.pi * (2.0 * n[None, :] + 1.0) * k[:, None] / (2.0 * N))


def _kernel_body(tc, x, w, bias, out):
    """x: [NHALF, 128, HCOLS] bf16 — half h of block (s,g) at [2*(s*NG+g)+h],
    cols (c_local, chin, b). w: [P, NSW*NCHUNK*NF] bf16 (lhsT chunks).
    bias: [OUT_F, 1] f32 (partition-major (s,o)). out: [OUT_F, BS] f32."""
    nc = tc.nc
    with ExitStack() as ctx:
        const_pool = ctx.enter_context(tc.tile_pool(name="const", bufs=1))
        xpool = ctx.enter_context(tc.tile_pool(name="xp", bufs=NHALF))
        upool = ctx.enter_context(tc.tile_pool(name="up", bufs=6))
        zpool = ctx.enter_context(tc.tile_pool(name="zp", bufs=2))
        opool = ctx.enter_context(tc.tile_pool(name="op", bufs=1))
        pft_pool = ctx.enter_context(tc.tile_pool(name="pft", bufs=1, space="PSUM"))

        # consts on the scalar HWDGE ring; x owns the sync ring
        w_sb = const_pool.tile([P, NSW * NCHUNK * NF], BF16)
        nc.scalar.dma_start(out=w_sb, in_=w)
        bias_sb = const_pool.tile([OUT_F, 1], F32)
        nc.scalar.dma_start(out=bias_sb, in_=bias)

        out_sb = opool.tile([OUT_F, BS], F32)
        psum_feat = [
            pft_pool.tile([NF, BS], F32, tag=f"feat{s}", name=f"psum_feat{s}")
            for s in range(NSW)
        ]

        xtiles = []
        for h in range(NHALF):
            t = xpool.tile([P, HCOLS], BF16, tag="x", name=f"x{h}")
            nc.sync.dma_start(out=t, in_=x[h])
            xtiles.append(t)

        for blk in range(NBLK):
            s, g = divmod(blk, NG)
            h0, h1 = xtiles[2 * blk], xtiles[2 * blk + 1]
            # pairwise c-tree, unit-stride bf16 adds (DVE 2x packed mode);
            # first-half partials overlap the second half's DMA
            u01 = upool.tile([P, GW], BF16, tag="u", name=f"u01_{blk}")
            nc.vector.tensor_add(out=u01, in0=h0[:, 0:GW], in1=h0[:, GW:2 * GW])
            u23 = upool.tile([P, GW], BF16, tag="u", name=f"u23_{blk}")
            nc.vector.tensor_add(out=u23, in0=h0[:, 2 * GW:3 * GW], in1=h0[:, 3 * GW:4 * GW])
            v0 = upool.tile([P, GW], BF16, tag="u", name=f"v0_{blk}")
            nc.vector.tensor_add(out=v0, in0=u01, in1=u23)
            u45 = upool.tile([P, GW], BF16, tag="u", name=f"u45_{blk}")
            nc.vector.tensor_add(out=u45, in0=h1[:, 0:GW], in1=h1[:, GW:2 * GW])
            u67 = upool.tile([P, GW], BF16, tag="u", name=f"u67_{blk}")
            nc.vector.tensor_add(out=u67, in0=h1[:, 2 * GW:3 * GW], in1=h1[:, 3 * GW:4 * GW])
            v1 = upool.tile([P, GW], BF16, tag="u", name=f"v1_{blk}")
            nc.vector.tensor_add(out=v1, in0=u45, in1=u67)
            z = zpool.tile([P, GW], BF16, tag="z", name=f"z_{blk}")
            nc.vector.tensor_add(out=z, in0=v0, in1=v1)

            # w-stationary matmuls: psum[o, b] += w_chunk.T @ z_chunk
            for chin in range(CPG):
                ch = g * CPG + chin
                nc.tensor.matmul(
                    psum_feat[s],
                    lhsT=w_sb[:, (s * NCHUNK + ch) * NF:(s * NCHUNK + ch + 1) * NF],
                    rhs=z[:, chin * P:(chin + 1) * P],
                    start=(ch == 0),
                    stop=(ch == NCHUNK - 1),
                )
            if g == NG - 1:
                # bias + LeakyReLU in one ACT instruction: Lrelu(psum + bias)
                nc.scalar.activation(
                    out_sb[s * NF:(s + 1) * NF, :],
                    psum_feat[s],
                    mybir.ActivationFunctionType.Lrelu,
                    bias=bias_sb[s * NF:(s + 1) * NF, :],
                    alpha=SLOPE,
                )
                nc.sync.dma_start(
                    out=out[s * NF:(s + 1) * NF, :],
                    in_=out_sb[s * NF:(s + 1) * NF, :],
                )


def _build():
    global _cached
    if _cached is not None:
        return _cached
    nc = bacc.Bacc(
        "TRN2",
        target_bir_lowering=False,
        debug=False,
        enable_asserts=False,
        num_devices=NCORES,
    )
    x_ap = nc.dram_tensor("x", [NHALF, P, HCOLS], BF16, kind="ExternalInput").ap()
    w_ap = nc.dram_tensor("w", [P, NSW * NCHUNK * NF], BF16, kind="ExternalInput").ap()
    b_ap = nc.dram_tensor("bias", [OUT_F, 1], F32, kind="ExternalInput").ap()
    out_ap = nc.dram_tensor("out", [OUT_F, BS], F32, kind="ExternalOutput").ap()
    with tile.TileContext(nc, trace_sim=False) as tc:
        _kernel_body(tc, x_ap, w_ap, b_ap, out_ap)
    nc.compile()
    _cached = nc
    return nc


def kernel(x, W, b):
    global last_results
    assert x.shape == (B, 1, NSW * NBINS * NDCT, HW, HW), x.shape
    nc = _build()

    # Host-side folding of the DCT matrices into the conv weights (tiny).
    Ct = _dct2(NDCT)                       # [f, t]
    Cs = _dct2(HW)                         # [p, i]
    Weff = np.einsum(
        "ft,pi,qj,sofpq->sotij", Ct, Cs, Cs, W.astype(np.float64), optimize=True
    ) / float(NBINS)
    Weff_k = Weff.reshape(NSW, NF, K)      # [s, o, k]
    # lhsT chunk layout: w[p, (s*NCHUNK+ch)*NF + o] = Weff_k[s, o, ch*128 + p]
    w_dev = np.ascontiguousarray(
        Weff_k.reshape(NSW, NF, NCHUNK, P).transpose(3, 0, 2, 1).reshape(P, NSW * NCHUNK * NF)
    ).astype(NP_BF16)
    bias_dev = np.ascontiguousarray(b.reshape(OUT_F, 1)).astype(np.float32)

    x2 = x.reshape(B, NSW, NBINS, NG, CPG, P)  # (b, s, c, g, chin, kin)
    in_maps = []
    for i in range(NCORES):
        xs = x2[i * BS:(i + 1) * BS]
        # -> [s, g, kin, c, chin, b]: contiguous [128, 4096] bf16 half-blocks
        xt = np.ascontiguousarray(xs.transpose(1, 3, 5, 2, 4, 0)).astype(NP_BF16)
        in_maps.append({
            "x": xt.reshape(NHALF, P, HCOLS),
            "w": w_dev,
            "bias": bias_dev,
        })
    res = run_bass_kernel_spmd(nc, in_maps, core_ids=list(range(NCORES)))
    last_results = res
    # device emits [s*64+o, b] per core; transpose back to [b, s*64+o]
    return np.concatenate([r["out"].T for r in res.results], axis=0)


# revision 7
# speedup vs baseline: 1.8020x; 1.0172x over previous
"""Trainium2 Bass kernel for nn_DCTFeatureModel.

Math: the reference pipeline (3D DCT-II over [time-in-bin, H, W], mean over
DCT bins, full-receptive-field Conv3d, bias, LeakyReLU) is linear up to the
LeakyReLU, so everything folds into a single small matmul:

    feat[b,s,o] = LeakyReLU( sum_{c,t,i,j} x[b,s,c,t,i,j] * Weff[s,o,t,i,j]
                             + bias[s,o] )
    Weff[s,o,t,i,j] = (1/8) * sum_{f,p,q} Ct[f,t] Cs[p,i] Cs[q,j] W[s,o,f,p,q]

Weff is tiny (2*64*2048 floats) and computed on host. The device kernel is
memory-bound: stream x (bf16, 8.4 MB/core), reduce over the 8 DCT bins (c),
then a [128b x 2048k] @ [2048k x 64o] matmul per subwindow.

Device dataflow (per core): host casts x to bf16 in blocks
[kin=128, (c, chin, b)] per (s, g) chunk-group; each block streams as two
1 MB half-DMAs (c0-3 / c4-7) on the sync HWDGE ring so halves land in
strict order. The c-reduction is a pairwise binary tree of unit-stride
bf16 tensor_adds on DVE (2x packed mode, ~0.6 us per [128,1024] add),
pipelined against the DMA stream; the half-split lets 3 of the 7 adds per
block run while the second half is still in flight. Matmuls are
w-stationary: lhsT = Weff chunk [128k, 64o], rhs = z chunk [128k, 128b],
PSUM [64o, 128b] per subwindow. Bias + LeakyReLU fuse into ONE scalar
engine activation (Lrelu, per-partition bias) straight out of PSUM, and
the output lands as [s*64+o, b] which the host transposes for free.

Sharding: pure data-parallel over batch, 1024/8 = 128 rows per core.
"""

from contextlib import ExitStack

import ml_dtypes
import numpy as np

import concourse.bacc as bacc
import concourse.tile as tile
from concourse import mybir
from concourse.bass_utils import run_bass_kernel_spmd

# Problem shapes (hardcoded per contract)
B = 1024
NCORES = 8
BS = B // NCORES          # 128 batch rows per core
NSW = 2                   # subwindows
NBINS = 8                 # DCT bins (mean-reduced)
NDCT = 32                 # time points per bin
HW = 8
NF = 64                   # conv output filters per subwindow
K = NDCT * HW * HW        # 2048 contraction elements per (s, c)
P = 128                   # partitions
NCHUNK = K // P           # 16 k-chunks of 128
NG = 2                    # chunk-groups per s
CPG = NCHUNK // NG        # 8 chunks per group
GW = CPG * P              # 1024 columns per (c) slice of a group
NBLK = NSW * NG           # 4 (s, g) blocks
NHALF = 2 * NBLK          # 8 half-block DMAs
HCOLS = (NBINS // 2) * GW  # 4096 columns per half (4 c-slices)
OUT_F = NSW * NF          # 128 output features
SLOPE = 0.02

F32 = mybir.dt.float32
BF16 = mybir.dt.bfloat16
NP_BF16 = ml_dtypes.bfloat16

_cached = None
last_results = None


def _dct2(N):
    n = np.arange(N, dtype=np.float64)
    k = np.arange(N, dtype=np.float64)
    return 2.0 * np.cos(np.pi * (2.0 * n[None, :] + 1.0) * k[:, None] / (2.0 * N))


def _kernel_body(tc, x, w, bias, out):
    """x: [NHALF, 128, HCOLS] bf16 — half h of block (s,g) at [2*(s*NG+g)+h],
    cols (c_local, chin, b). w: [P, NSW*NCHUNK*NF] bf16 (lhsT chunks).
    bias: [OUT_F, 1] f32 (partition-major (s,o)). out: [OUT_F, BS] f32."""
    nc = tc.nc
    with ExitStack() as ctx:
        const_pool = ctx.enter_context(tc.tile_pool(name="const", bufs=1))
        xpool = ctx.enter_context(tc.tile_pool(name="xp", bufs=NHALF))
        upool = ctx.enter_context(tc.tile_pool(name="up", bufs=6))
        zpool = ctx.enter_context(tc.tile_pool(name="zp", bufs=2))
        opool = ctx.enter_context(tc.tile_pool(name="op", bufs=1))
        pft_pool = ctx.enter_context(tc.tile_pool(name="pft", bufs=1, space="PSUM"))

        # consts on the scalar HWDGE ring; x owns the sync ring
        w_sb = const_pool.tile([P, NSW * NCHUNK * NF], BF16)
        nc.scalar.dma_start(out=w_sb, in_=w)
        bias_sb = const_pool.tile([OUT_F, 1], F32)
        nc.scalar.dma_start(out=bias_sb, in_=bias)

        out_sb = opool.tile([OUT_F, BS], F32)
        psum_feat = [
            pft_pool.tile([NF, BS], F32, tag=f"feat{s}", name=f"psum_feat{s}")
            for s in range(NSW)
        ]

        # blocks 0-2: symmetric 1 MB halves (c0-3 | c4-7). Last block: c0-5 in
        # the first DMA, c6-7 in the second, so only one lvl0 add remains on
        # the post-stream critical path.
        splits = [6 * GW if blk == NBLK - 1 else 4 * GW for blk in range(NBLK)]
        xtiles = []
        for blk in range(NBLK):
            cut = splits[blk]
            ta = xpool.tile([P, cut], BF16, tag="xa", name=f"xa{blk}")
            nc.sync.dma_start(out=ta, in_=x[blk][:, 0:cut])
            tb = xpool.tile([P, NBINS * GW - cut], BF16, tag="xb", name=f"xb{blk}")
            nc.sync.dma_start(out=tb, in_=x[blk][:, cut:])
            xtiles.append((ta, tb))

        for blk in range(NBLK):
            s, g = divmod(blk, NG)
            h0, h1 = xtiles[blk]
            last = blk == NBLK - 1
            # pairwise c-tree, unit-stride bf16 adds (DVE 2x packed mode);
            # first-DMA partials overlap the second DMA
            u01 = upool.tile([P, GW], BF16, tag="u", name=f"u01_{blk}")
            nc.vector.tensor_add(out=u01, in0=h0[:, 0:GW], in1=h0[:, GW:2 * GW])
            u23 = upool.tile([P, GW], BF16, tag="u", name=f"u23_{blk}")
            nc.vector.tensor_add(out=u23, in0=h0[:, 2 * GW:3 * GW], in1=h0[:, 3 * GW:4 * GW])
            v0 = upool.tile([P, GW], BF16, tag="u", name=f"v0_{blk}")
            nc.vector.tensor_add(out=v0, in0=u01, in1=u23)
            u45 = upool.tile([P, GW], BF16, tag="u", name=f"u45_{blk}")
            if last:
                # c4, c5 arrived with the first DMA; fold them in pre-stream-end
                nc.vector.tensor_add(out=u45, in0=h0[:, 4 * GW:5 * GW], in1=h0[:, 5 * GW:6 * GW])
                v0b = upool.tile([P, GW], BF16, tag="u", name=f"v0b_{blk}")
                nc.vector.tensor_add(out=v0b, in0=v0, in1=u45)
                u67 = upool.tile([P, GW], BF16, tag="u", name=f"u67_{blk}")
                nc.vector.tensor_add(out=u67, in0=h1[:, 0:GW], in1=h1[:, GW:2 * GW])
                va, vb = v0b, u67
            else:
                nc.vector.tensor_add(out=u45, in0=h1[:, 0:GW], in1=h1[:, GW:2 * GW])
                u67 = upool.tile([P, GW], BF16, tag="u", name=f"u67_{blk}")
                nc.vector.tensor_add(out=u67, in0=h1[:, 2 * GW:3 * GW], in1=h1[:, 3 * GW:4 * GW])
                v1 = upool.tile([P, GW], BF16, tag="u", name=f"v1_{blk}")
                nc.vector.tensor_add(out=v1, in0=u45, in1=u67)
                va, vb = v0, v1
            z = zpool.tile([P, GW], BF16, tag="z", name=f"z_{blk}")
            # final add col-split so the first 4 matmuls overlap the second add
            nhalfz = 2 if last else 1
            step = GW // nhalfz
            for zh in range(nhalfz):
                nc.vector.tensor_add(
                    out=z[:, zh * step:(zh + 1) * step],
                    in0=va[:, zh * step:(zh + 1) * step],
                    in1=vb[:, zh * step:(zh + 1) * step],
                )
                # w-stationary matmuls: psum[o, b] += w_chunk.T @ z_chunk
                for j in range(step // P):
                    chin = zh * (step // P) + j
                    ch = g * CPG + chin
                    nc.tensor.matmul(
                        psum_feat[s],
                        lhsT=w_sb[:, (s * NCHUNK + ch) * NF:(s * NCHUNK + ch + 1) * NF],
                        rhs=z[:, chin * P:(chin + 1) * P],
                        start=(ch == 0),
                        stop=(ch == NCHUNK - 1),
                    )
            if g == NG - 1:
                # bias + LeakyReLU in one ACT instruction: Lrelu(psum + bias)
                nc.scalar.activation(
                    out_sb[s * NF:(s + 1) * NF, :],
                    psum_feat[s],
                    mybir.ActivationFunctionType.Lrelu,
                    bias=bias_sb[s * NF:(s + 1) * NF, :],
                    alpha=SLOPE,
                )
                nc.sync.dma_start(
                    out=out[s * NF:(s + 1) * NF, :],
                    in_=out_sb[s * NF:(s + 1) * NF, :],
                )


def _build():
    global _cached
    if _cached is not None:
        return _cached
    nc = bacc.Bacc(
        "TRN2",
        target_bir_lowering=False,
        debug=False,
        enable_asserts=False,
        num_devices=NCORES,
    )
    x_ap = nc.dram_tensor("x", [NBLK, P, NBINS * GW], BF16, kind="ExternalInput").ap()
    w_ap = nc.dram_tensor("w", [P, NSW * NCHUNK * NF], BF16, kind="ExternalInput").ap()
    b_ap = nc.dram_tensor("bias", [OUT_F, 1], F32, kind="ExternalInput").ap()
    out_ap = nc.dram_tensor("out", [OUT_F, BS], F32, kind="ExternalOutput").ap()
    with tile.TileContext(nc, trace_sim=False) as tc:
        _kernel_body(tc, x_ap, w_ap, b_ap, out_ap)
    nc.compile()
    _cached = nc
    return nc


def kernel(x, W, b):
    global last_results
    assert x.shape == (B, 1, NSW * NBINS * NDCT, HW, HW), x.shape
    nc = _build()

    # Host-side folding of the DCT matrices into the conv weights (tiny).
    Ct = _dct2(NDCT)                       # [f, t]
    Cs = _dct2(HW)                         # [p, i]
    Weff = np.einsum(
        "ft,pi,qj,sofpq->sotij", Ct, Cs, Cs, W.astype(np.float64), optimize=True
    ) / float(NBINS)
    Weff_k = Weff.reshape(NSW, NF, K)      # [s, o, k]
    # lhsT chunk layout: w[p, (s*NCHUNK+ch)*NF + o] = Weff_k[s, o, ch*128 + p]
    w_dev = np.ascontiguousarray(
        Weff_k.reshape(NSW, NF, NCHUNK, P).transpose(3, 0, 2, 1).reshape(P, NSW * NCHUNK * NF)
    ).astype(NP_BF16)
    bias_dev = np.ascontiguousarray(b.reshape(OUT_F, 1)).astype(np.float32)

    x2 = x.reshape(B, NSW, NBINS, NG, CPG, P)  # (b, s, c, g, chin, kin)
    in_maps = []
    for i in range(NCORES):
        xs = x2[i * BS:(i + 1) * BS]
        # -> [s, g, kin, c, chin, b]: contiguous [128, 4096] bf16 half-blocks
        xt = np.ascontiguousarray(xs.transpose(1, 3, 5, 2, 4, 0)).astype(NP_BF16)
        in_maps.append({
            "x": xt.reshape(NBLK, P, NBINS * GW),
            "w": w_dev,
            "bias": bias_dev,
        })
    res = run_bass_kernel_spmd(nc, in_maps, core_ids=list(range(NCORES)))
    last_results = res
    # device emits [s*64+o, b] per core; transpose back to [b, s*64+o]
    return np.concatenate([r["out"].T for r in res.results], axis=0)
